# revision 5
# baseline (speedup 1.0000x reference)
"""Two-layer GCN encoder on 8 Trainium2 NeuronCores.

Strategy (dst-partitioned, matmul-based segment sum):
  - Nodes are grouped into 392 blocks of 128; blocks are assigned to
    (core, slot) pairs balancing edge counts, 49 slots per core.
  - Every edge is owned by the core owning its dst block, so each core's
    aggregation for its blocks is complete: no all-reduce needed.
  - Per edge tile (128 edges): gather x[src] rows into SBUF partitions via
    dma_gather, build P[e, n] = (iota == dstcol) * w with one fused DVE
    tensor_scalar, and accumulate aggT[feat, node] += Xg.T @ P in PSUM.
  - Per block: h = relu(aggT.T @ W + b) via two matmuls (bias as a K=1
    matmul) and an ACT relu eviction.
  - One AllGather shares the layer-1 activations; layer 2 repeats the
    same pipeline reading the gathered activations.

dma_gather uses int16 indices, so gather sources are split at row 32768
(lo/hi) and each block's edges are bucketed per layer accordingly.
"""

import numpy as np
from concourse import bacc, mybir, tile
from concourse.bass_utils import run_bass_kernel_spmd

P = 128
N_NODES = 50000
N_EDGES = 800000
NFEAT = 128
NC = 8
SLOTS = 49                 # node blocks per core
NB = NC * SLOTS            # 392 blocks, 50176 padded rows
SHARD = SLOTS * P          # 6272 rows per core
NFULL = NB * P             # 50176
LO_SPLIT = 32768           # int16 index limit for dma_gather
GROUP = 5                  # slots per gather group

FP = mybir.dt.float32

# Set by kernel() for test harness introspection (trace results etc.)
last_run_results = None


def _wrap16(flat):
    """dma_gather index layout: logical i -> [i % 16, i // 16], x8 replicated."""
    n16 = len(flat) // 16
    arr = np.asarray(flat, dtype=np.int16).reshape(n16, 16).T  # [16, n16]
    return np.tile(arr, (8, 1))  # [128, n16]


def _prep(edge_index, edge_weight):
    """Host-side sharding: block assignment, per-layer gather indices, colw."""
    src = edge_index[0].astype(np.int64)
    dst = edge_index[1].astype(np.int64)
    w = edge_weight.astype(np.float32)

    blk = dst >> 7
    col = (dst & 127).astype(np.float32)

    cnt = np.bincount(blk, minlength=NB)
    order = np.argsort(-cnt, kind="stable")
    block_at = order.reshape(SLOTS, NC).T          # [core, slot] -> block
    core_of = np.empty(NB, np.int64)
    slot_of = np.empty(NB, np.int64)
    for c in range(NC):
        for s in range(SLOTS):
            core_of[block_at[c, s]] = c
            slot_of[block_at[c, s]] = s

    eorder = np.argsort(blk, kind="stable")
    estart = np.zeros(NB + 1, np.int64)
    np.cumsum(cnt, out=estart[1:])

    # per-layer gather index value for each edge
    sblk = src >> 7
    ag_row = core_of[sblk] * SHARD + slot_of[sblk] * P + (src & 127)
    vals = [src, ag_row]

    groups = [list(range(g, min(g + GROUP, SLOTS))) for g in range(0, SLOTS, GROUP)]

    # per (layer, core, slot): lo/hi edge id lists
    ids_l = [[[None] * SLOTS for _ in range(NC)] for _ in range(2)]
    nt_l = []  # per layer: (LT[s], HT[s]) shared tile schedule
    for l in range(2):
        v = vals[l]
        LT = np.zeros(SLOTS, np.int64)
        HT = np.zeros(SLOTS, np.int64)
        for c in range(NC):
            for s in range(SLOTS):
                b = block_at[c, s]
                ids = eorder[estart[b]:estart[b + 1]]
                m = v[ids] < LO_SPLIT
                lo, hi = ids[m], ids[~m]
                ids_l[l][c][s] = (lo, hi)
                LT[s] = max(LT[s], (len(lo) + P - 1) // P)
                HT[s] = max(HT[s], (len(hi) + P - 1) // P)
        nt_l.append((LT, HT))

    # Build flat per-call gather index arrays + colw, and the static schedule.
    # Tile enumeration: for g in groups: for part in (lo, hi): for s in g:
    #   tiles of (part, s).  Calls: one per (group, part).
    sched = []            # per layer: list of group dicts
    idx_np = [[], []]     # per layer per core: wrapped int16 [128, n16]
    colw_np = [[], []]    # per layer per core: [128, 2*ntiles] f32
    for l in range(2):
        LT, HT = nt_l[l]
        v = vals[l]
        gdescs = []
        tid0 = 0
        for g in groups:
            lo_tiles = int(sum(LT[s] for s in g))
            hi_tiles = int(sum(HT[s] for s in g))
            gdescs.append({
                "slots": g,
                "lo_tiles": lo_tiles,
                "hi_tiles": hi_tiles,
                "tid0": tid0,
            })
            tid0 += lo_tiles + hi_tiles
        ntiles = tid0
        sched.append({"LT": LT, "HT": HT, "groups": gdescs, "ntiles": ntiles})

        for c in range(NC):
            flat_idx = []
            colw = np.zeros((P, 2 * ntiles), np.float32)
            tid = 0
            for g in groups:
                for part in range(2):
                    T = LT if part == 0 else HT
                    for s in g:
                        lo, hi = ids_l[l][c][s]
                        ids = lo if part == 0 else hi
                        n = int(T[s]) * P
                        iv = np.zeros(n, np.int64)
                        cv = np.zeros(n, np.float32)
                        wv = np.zeros(n, np.float32)
                        iv[:len(ids)] = v[ids] - (0 if part == 0 else LO_SPLIT)
                        cv[:len(ids)] = col[ids]
                        wv[:len(ids)] = w[ids]
                        flat_idx.append(iv)
                        for t in range(int(T[s])):
                            colw[:, 2 * tid] = cv[t * P:(t + 1) * P]
                            colw[:, 2 * tid + 1] = wv[t * P:(t + 1) * P]
                            tid += 1
            idx_np[l].append(_wrap16(np.concatenate(flat_idx)))
            colw_np[l].append(colw)

    return block_at, sched, idx_np, colw_np


def _build(sched, n16, x_rows):
    """Build the SPMD bass program. Returns finalized nc."""
    nc = bacc.Bacc(num_devices=NC)

    x_in = nc.declare_dram_parameter("x", [x_rows, NFEAT], FP, isOutput=False)
    w1_in = nc.declare_dram_parameter("W1", [NFEAT, NFEAT], FP, isOutput=False)
    w2_in = nc.declare_dram_parameter("W2", [NFEAT, NFEAT], FP, isOutput=False)
    b1_in = nc.declare_dram_parameter("b1", [1, NFEAT], FP, isOutput=False)
    b2_in = nc.declare_dram_parameter("b2", [1, NFEAT], FP, isOutput=False)
    iota_in = nc.declare_dram_parameter("iota", [P, P], FP, isOutput=False)
    idx_in = [
        nc.declare_dram_parameter(f"idx{l}", [P, n16[l]], mybir.dt.int16, isOutput=False)
        for l in range(2)
    ]
    colw_in = [
        nc.declare_dram_parameter(
            f"colw{l}", [P, 2 * sched[l]["ntiles"]], FP, isOutput=False
        )
        for l in range(2)
    ]
    out = nc.declare_dram_parameter("out", [SHARD, NFEAT], FP, isOutput=True)

    relu = mybir.ActivationFunctionType.Relu

    with tile.TileContext(nc) as tc:
        with tc.tile_pool(name="const", bufs=1) as cpool, \
             tc.tile_pool(name="gbuf", bufs=2) as gpool, \
             tc.tile_pool(name="pmat", bufs=8) as ppool, \
             tc.tile_pool(name="evict", bufs=3) as epool, \
             tc.tile_pool(name="hout", bufs=3) as hpool, \
             tc.tile_pool(name="psA", bufs=2, space="PSUM") as psA, \
             tc.tile_pool(name="psB", bufs=2, space="PSUM") as psB, \
             tc.tile_pool(name="dram", bufs=1, space="DRAM") as dpool:

            iota_t = cpool.tile([P, P], FP)
            w_t = [cpool.tile([P, P], FP, name=f"w{l}") for l in range(2)]
            b_t = [cpool.tile([1, P], FP, name=f"b{l}") for l in range(2)]
            ones_t = cpool.tile([1, P], FP)
            idx_t = [cpool.tile([P, n16[l]], mybir.dt.int16, name=f"idx{l}") for l in range(2)]
            colw_t = [cpool.tile([P, 2 * sched[l]["ntiles"]], FP, name=f"colw{l}") for l in range(2)]

            nc.sync.dma_start(out=iota_t[:], in_=iota_in[:])
            nc.sync.dma_start(out=w_t[0][:], in_=w1_in[:])
            nc.sync.dma_start(out=w_t[1][:], in_=w2_in[:])
            nc.sync.dma_start(out=b_t[0][:], in_=b1_in[:])
            nc.sync.dma_start(out=b_t[1][:], in_=b2_in[:])
            nc.vector.memset(ones_t[:], 1.0)
            for l in range(2):
                nc.sync.dma_start(out=idx_t[l][:], in_=idx_in[l][:])
                nc.sync.dma_start(out=colw_t[l][:], in_=colw_in[l][:])

            h_shard = dpool.tile([SHARD, NFEAT], FP, name="h_shard")
            h_full = dpool.tile([NFULL, NFEAT], FP, name="h_full")

            def layer(l, src_lo, src_hi, dst_ap):
                LT, HT = sched[l]["LT"], sched[l]["HT"]
                i16 = 0  # running int16-column offset into idx_t[l]
                for gd in sched[l]["groups"]:
                    slots = gd["slots"]
                    lo_tiles, hi_tiles = gd["lo_tiles"], gd["hi_tiles"]
                    all_tiles = lo_tiles + hi_tiles
                    gbuf = gpool.tile([P, all_tiles * P], FP, name="gbuf", tag="gbuf")
                    for part, (ntile, srcap) in enumerate(
                        [(lo_tiles, src_lo), (hi_tiles, src_hi)]
                    ):
                        # dma_gather hangs above ~1024 indices per call;
                        # split each part into <=8-tile calls.
                        base = 0 if part == 0 else lo_tiles * P
                        t0 = 0
                        while t0 < ntile:
                            nt = min(8, ntile - t0)
                            nidx = nt * P
                            o = base + t0 * P
                            nc.gpsimd.dma_gather(
                                out_ap=gbuf[:, o:o + nidx].rearrange(
                                    "p (t e) -> p t e", e=P
                                ),
                                in_ap=srcap,
                                idxs_ap=idx_t[l][:, i16:i16 + nidx // 16],
                                num_idxs=nidx,
                                num_idxs_reg=nidx,
                                elem_size=P,
                            )
                            i16 += nidx // 16
                            t0 += nt
                    # per-slot tile ranges within gbuf; colw tile id for gbuf
                    # tile gt is tid0 + gt (same (part, slot) enumeration)
                    lo_base = 0
                    hi_base = lo_tiles
                    for s in slots:
                        nlo, nhi = int(LT[s]), int(HT[s])
                        tlist = [lo_base + t for t in range(nlo)] + \
                                [hi_base + t for t in range(nhi)]
                        lo_base += nlo
                        hi_base += nhi
                        ntot = nlo + nhi
                        aggT = psA.tile([P, P], FP, space="PSUM", name="aggT", tag="aggT")
                        for k, gt in enumerate(tlist):
                            tid = gd["tid0"] + gt
                            pm = ppool.tile([P, P], FP, name="pm", tag="pm")
                            nc.vector.tensor_scalar(
                                out=pm[:],
                                in0=iota_t[:],
                                scalar1=colw_t[l][:, 2 * tid:2 * tid + 1],
                                scalar2=colw_t[l][:, 2 * tid + 1:2 * tid + 2],
                                op0=mybir.AluOpType.is_equal,
                                op1=mybir.AluOpType.mult,
                            )
                            nc.tensor.matmul(
                                out=aggT[:],
                                lhsT=gbuf[:, gt * P:(gt + 1) * P],
                                rhs=pm[:],
                                start=(k == 0),
                                stop=(k == ntot - 1),
                            )
                        aggT_sb = epool.tile([P, P], FP, name="evict", tag="evict")
                        nc.scalar.copy(out=aggT_sb[:], in_=aggT[:])
                        h_ps = psB.tile([P, P], FP, space="PSUM", name="hps", tag="hps")
                        nc.tensor.matmul(
                            out=h_ps[:], lhsT=aggT_sb[:], rhs=w_t[l][:],
                            start=True, stop=False,
                        )
                        nc.tensor.matmul(
                            out=h_ps[:], lhsT=ones_t[0:1, :], rhs=b_t[l][0:1, :],
                            start=False, stop=True,
                        )
                        h_sb = hpool.tile([P, P], FP, name="hout", tag="hout")
                        nc.scalar.activation(out=h_sb[:], in_=h_ps[:], func=relu)
                        nc.sync.dma_start(
                            out=dst_ap[s * P:(s + 1) * P, :], in_=h_sb[:]
                        )

            layer(0, x_in[0:LO_SPLIT, :], x_in[LO_SPLIT:N_NODES, :], h_shard[:])

            nc.gpsimd.collective_compute(
                "AllGather",
                mybir.AluOpType.bypass,
                replica_groups=[list(range(NC))],
                ins=[h_shard[:]],
                outs=[h_full[:]],
            )

            layer(1, h_full[0:LO_SPLIT, :], h_full[LO_SPLIT:NFULL, :], out[:])

    nc.finalize()
    return nc


def kernel(x, edge_index, edge_weight, W1, b1, W2, b2):
    global last_run_results
    x = np.ascontiguousarray(np.asarray(x, dtype=np.float32))
    edge_index = np.asarray(edge_index)
    edge_weight = np.asarray(edge_weight, dtype=np.float32)

    block_at, sched, idx_np, colw_np = _prep(edge_index, edge_weight)
    n16 = [idx_np[l][0].shape[1] for l in range(2)]
    nc = _build(sched, n16, x.shape[0])

    iota_np = np.broadcast_to(
        np.arange(P, dtype=np.float32), (P, P)
    ).copy()
    in_maps = []
    for c in range(NC):
        in_maps.append({
            "x": x,
            "W1": np.ascontiguousarray(W1, dtype=np.float32),
            "W2": np.ascontiguousarray(W2, dtype=np.float32),
            "b1": np.ascontiguousarray(b1, dtype=np.float32).reshape(1, NFEAT),
            "b2": np.ascontiguousarray(b2, dtype=np.float32).reshape(1, NFEAT),
            "iota": iota_np,
            "idx0": idx_np[0][c],
            "idx1": idx_np[1][c],
            "colw0": colw_np[0][c],
            "colw1": colw_np[1][c],
        })

    import os
    trace = bool(int(os.environ.get("GCN_TRACE", "0")))
    res = run_bass_kernel_spmd(
        nc, in_maps, list(range(NC)), trace=trace,
    )
    last_run_results = res

    full = np.zeros((NFULL, NFEAT), np.float32)
    for c in range(NC):
        shard = res.results[c]["out"]
        for s in range(SLOTS):
            b = int(block_at[c, s])
            full[b * P:(b + 1) * P] = shard[s * P:(s + 1) * P]
    return full[:N_NODES]
